# revision 16
# baseline (speedup 1.0000x reference)
"""TRN2 Bass kernel for nn_Block1_43542378447225.

Pipeline (per sample, one NeuronCore; batch=2 -> cores 0/1 do real work):
  conv1 -> relu -> conv2 -> relu -> Hopfield(z2) -> D
  backward (w2b matmul, mask, Scomb matmul) -> C  (e_sum in composite-window form)
  blocked e_min via permutation matmuls -> mask -> masked patch forward -> z2_masked
  Hopfield(z2_masked) -> output

Host precomputes im2col patches (P1 [48,256], X [100,192]) and weight layouts,
packed into few DMA-able blocks; the device does all matmuls/softmax/masking.

Layout conventions:
  pq = p*8+q (64 output positions), uv = u*10+v (100 composite-window offsets)
  chunk t = conv2 kernel row kr, a = conv2 kernel col ks
  kc = t*128 + a*32 + c1 (hidden index, 4 chunks of 128 partitions)
"""
import numpy as np

import concourse.bass as bass
import concourse.bacc as bacc
import concourse.mybir as mybir
import concourse.tile as tile
from concourse.bass_utils import run_bass_kernel_spmd

F32 = mybir.dt.float32
BF16 = mybir.dt.bfloat16
AF = mybir.ActivationFunctionType
ALU = mybir.AluOpType

N_CORES = 8
BETA = 0.125  # 1/sqrt(64)
BIG = 1.0e30

_CACHE = {}


# ---------------------------------------------------------------- host prep
def _build_scomb_w1big(w1):
    w1s = w1.sum(axis=1)
    Scomb = np.zeros((4, 32, 4, 100), np.float32)  # [a, c1, t, uv]
    W1big = np.zeros((100, 3, 4, 4, 32), np.float32)  # [uv, h, t, a, c1]
    for t in range(4):
        for a in range(4):
            for u in range(10):
                ki = u - 2 * t
                if not (0 <= ki < 4):
                    continue
                for v in range(10):
                    kj = v - 2 * a
                    if not (0 <= kj < 4):
                        continue
                    Scomb[a, :, t, u * 10 + v] = w1s[:, ki, kj]
                    W1big[u * 10 + v, :, t, a, :] = w1[:, :, ki, kj].T
    # partition index = a*32+c1 -> merge (a, c1); free = t*100+uv
    Scomb = Scomb.reshape(128, 400)
    W1big = W1big.reshape(100, 1536)
    return Scomb, W1big


def _host_prep(w1, b1, w2, b2, K, Vw):
    # wA [128, 897]: w2fT | b2 | KT | Vw | ident
    wA = np.zeros((128, 897), np.float32)
    wA[:, 0:256] = np.transpose(w2, (3, 1, 2, 0)).reshape(128, 256)  # w2fT
    wA[0:64, 256:257] = b2[:, None]
    wA[0:64, 257:769] = K.T
    wA[0:64, 769:833] = Vw
    wA[0:64, 833:897] = np.eye(64, dtype=np.float32)

    Scomb, W1big = _build_scomb_w1big(w1)
    PermF = np.zeros((100, 9, 16), np.float32)
    for k in range(9):
        dp, dq = k // 3 - 1, k % 3 - 1
        for im in range(4):
            u = 4 * dp + im + 3
            if not (0 <= u < 10):
                continue
            for jm in range(4):
                v = 4 * dq + jm + 3
                if not (0 <= v < 10):
                    continue
                PermF[u * 10 + v, k, im * 4 + jm] = 1.0
    CandM = np.zeros((100, 3, 128), np.float32)
    for k in range(9):
        cc, kk = divmod(k, 4)
        CandM[:, cc, kk * 32:kk * 32 + 16] = PermF[:, k, :]
    PermB = np.transpose(PermF, (2, 1, 0)).reshape(16, 900)

    # wB [128, 2196]: w2b | Scomb | CandM_ext | PermB
    wB = np.zeros((128, 2196), np.float32)
    wB[0:64, 0:512] = 2.0 * np.transpose(w2, (0, 2, 3, 1)).reshape(64, 512)
    wB[:, 512:912] = Scomb
    wB[0:100, 912:1296] = CandM.reshape(100, 384)
    wB[0:16, 1296:2196] = PermB

    # wC [128, 1536]: W1big rows 0:100
    wC = np.zeros((128, 1536), np.float32)
    wC[0:100, :] = W1big

    return {"wA": wA, "wB": wB, "wC": wC,
            "_w1f": np.ascontiguousarray(np.transpose(w1, (2, 3, 1, 0)).reshape(48, 32)),
            "_b1": np.ascontiguousarray(b1[:, None])}


def _sample_prep(x_s, w1f, b1c):
    xp1 = np.pad(x_s, ((0, 0), (1, 1), (1, 1)))
    xp3 = np.pad(x_s, ((0, 0), (3, 3), (3, 3)))
    P1 = np.zeros((4, 4, 3, 16, 16), np.float32)
    for kr in range(4):
        for ks in range(4):
            P1[kr, ks] = xp1[:, kr:kr + 32:2, ks:ks + 32:2][:, :16, :16]
    X = np.zeros((10, 10, 3, 8, 8), np.float32)
    for u in range(10):
        for v in range(10):
            X[u, v] = xp3[:, u:u + 32:4, v:v + 32:4][:, :8, :8]
    smpl = np.zeros((100, 481), np.float32)
    smpl[0:48, 0:256] = P1.reshape(48, 256)
    smpl[:, 256:448] = X.reshape(100, 192)
    smpl[0:48, 448:480] = w1f
    smpl[0:32, 480:481] = b1c
    return smpl


# ---------------------------------------------------------------- device build
def _hopfield(nc, sb, ps, z_sb, KT_bf, KV_sb, ident_sb, tag):
    """z_sb [64(c), 64(pq)] -> returns q_ps [64(c), 64(pq)] PSUM tile.
    Scores are bounded (|beta*S| small), so softmax skips max subtraction."""
    z_bf = sb.tile([64, 64], BF16, tag=f"zbf{tag}", name=f"zbf{tag}")
    nc.vector.tensor_copy(out=z_bf[:], in_=z_sb)
    S_ps = ps.tile([64, 512], F32, tag="S", bufs=1, name=f"S{tag}")
    nc.tensor.matmul(S_ps[:], z_bf[:], KT_bf, start=True, stop=True)
    att = sb.tile([64, 512], F32, tag=f"att{tag}", name=f"att{tag}")
    ssum = sb.tile([64, 1], F32, tag=f"ssum{tag}", name=f"ssum{tag}")
    nc.scalar.activation(out=att[:], in_=S_ps[:], func=AF.Exp,
                         bias=0.0, scale=BETA, accum_out=ssum[:])
    rec = sb.tile([64, 1], F32, tag=f"rec{tag}", name=f"rec{tag}")
    nc.vector.reciprocal(rec[:], ssum[:])
    nc.vector.tensor_scalar_mul(att[:], att[:], rec[:])
    attT = sb.tile([128, 4, 64], F32, tag=f"attT{tag}", name=f"attT{tag}")
    for t in range(4):
        tr_ps = ps.tile([128, 64], F32, tag="g128", bufs=4, name=f"tr{tag}{t}")
        nc.tensor.transpose(tr_ps[:], att[:, t * 128:(t + 1) * 128], ident_sb)
        if t % 2 == 0:
            nc.vector.tensor_copy(out=attT[:, t, :], in_=tr_ps[:])
        else:
            nc.scalar.copy(out=attT[:, t, :], in_=tr_ps[:])
    q_ps = ps.tile([64, 64], F32, tag="q64", bufs=2, name=f"q{tag}")
    for t in range(4):
        nc.tensor.matmul(q_ps[:], KV_sb[:, t, :], attT[:, t, :],
                         start=(t == 0), stop=(t == 3))
    return q_ps


def _build_nc(debug=False):
    nc = bacc.Bacc("TRN2", target_bir_lowering=False, debug=False,
                   num_devices=N_CORES)
    d_smpl = nc.dram_tensor("smpl", [100, 481], F32, kind="ExternalInput")
    d_wA = nc.dram_tensor("wA", [128, 897], F32, kind="ExternalInput")
    d_wB = nc.dram_tensor("wB", [128, 2196], F32, kind="ExternalInput")
    d_wC = nc.dram_tensor("wC", [128, 1536], F32, kind="ExternalInput")
    out_t = nc.dram_tensor("out", [64, 64], F32, kind="ExternalOutput")
    probes = {}

    def probe(name, shape):
        if debug:
            probes[name] = nc.dram_tensor("probe_" + name, shape, F32,
                                          kind="ExternalOutput")
        return probes.get(name)

    with tile.TileContext(nc) as tc:
        with tc.tile_pool(name="sb", bufs=1) as sb, \
             tc.tile_pool(name="ps", bufs=1, space="PSUM") as ps:
            # ---- loads: 2 HWDGE queues (SP: smpl+wB, ACT: wA+wC)
            smpl = sb.tile([100, 481], F32, tag="smpl")
            nc.sync.dma_start(out=smpl[:], in_=d_smpl[:])
            wA = sb.tile([128, 897], F32, tag="wA")
            nc.scalar.dma_start(out=wA[:], in_=d_wA[:])
            wB = sb.tile([128, 2196], F32, tag="wB")
            nc.sync.dma_start(out=wB[:], in_=d_wB[:])
            wC = sb.tile([128, 1536], F32, tag="wC")
            nc.scalar.dma_start(out=wC[:], in_=d_wC[:])
            C_ext = sb.tile([100, 64], F32, tag="C_ext")

            P1 = smpl[0:48, 0:256]
            X = smpl[:, 256:448].rearrange("u (h q) -> u h q", h=3)
            w1f = smpl[0:48, 448:480]
            b1 = smpl[0:32, 480:481]
            w2fT = wA[:, 0:256].rearrange("k (t c) -> k t c", t=4)
            b2 = wA[0:64, 256:257]
            KT = wA[0:64, 257:769]
            Vw = wA[0:64, 769:833]
            ident = wA[0:64, 833:897]
            w2b = wB[0:64, 0:512]
            Scomb = wB[:, 512:912].rearrange("k (t u) -> k t u", t=4)
            CandM = wB[0:100, 912:1296].rearrange("u (c k) -> u c k", c=3)
            PermB = wB[0:16, 1296:2196]
            W1big = wC[0:100, :].rearrange("u (h t k) -> u h t k", h=3, t=4)

            # ---- conv1 + relu into padded a1p [32, 18, 18]
            a1_ps = ps.tile([32, 256], F32, tag="a1", bufs=1)
            nc.tensor.matmul(a1_ps[:], w1f, P1, start=True, stop=True)
            a1p = sb.tile([32, 18, 18], F32, tag="a1p")
            nc.vector.memset(a1p[:], 0.0)
            nc.scalar.activation(
                out=a1p[:, 1:17, 1:17],
                in_=a1_ps[:].rearrange("c (p q) -> c p q", p=16),
                func=AF.Relu, bias=b1, scale=1.0)

            # ---- KV = K @ Vw chunked [128, 4, 64]; KT in bf16 for scores
            KT_bf = sb.tile([64, 512], BF16, tag="KT_bf")
            nc.vector.tensor_copy(out=KT_bf[:], in_=KT)
            KV = sb.tile([128, 4, 64], F32, tag="KV")
            for t in range(4):
                kv_ps = ps.tile([128, 64], F32, tag="g128", bufs=4,
                                name=f"kv{t}")
                nc.tensor.matmul(kv_ps[:], KT[:, t * 128:(t + 1) * 128],
                                 Vw, start=True, stop=True)
                nc.scalar.copy(out=KV[:, t, :], in_=kv_ps[:])

            # ---- P2 im2col: P2[a*32+c1, t, p, q] = a1p[c1, 2p+t, 2q+a]
            P2 = sb.tile([128, 4, 8, 8], F32, tag="P2")
            a1p_ap = a1p[:]
            for a in range(4):
                src = bass.AP(
                    tensor=a1p_ap.tensor,
                    offset=a1p_ap.offset + a,
                    ap=[[324, 32], [18, 4], [36, 8], [2, 8]])
                eng = nc.vector if a < 3 else nc.gpsimd
                eng.tensor_copy(out=P2[a * 32:(a + 1) * 32, :, :, :], in_=src)
            P2f = P2[:].rearrange("k t p q -> k t (p q)")
            M1W = sb.tile([128, 4, 64], F32, tag="M1W")
            nc.vector.tensor_scalar(out=M1W[:], in0=P2f, scalar1=0.0,
                                    scalar2=None, op0=ALU.not_equal)

            # ---- conv2 + relu -> z2 [64, 64], m2
            z2_ps = ps.tile([64, 64], F32, tag="q64", bufs=2)
            for t in range(4):
                nc.tensor.matmul(z2_ps[:], w2fT[:, t, :], P2f[:, t, :],
                                 start=(t == 0), stop=(t == 3))
            z2 = sb.tile([64, 64], F32, tag="z2")
            nc.scalar.activation(out=z2[:], in_=z2_ps[:], func=AF.Relu,
                                 bias=b2, scale=1.0)
            m2 = sb.tile([64, 64], F32, tag="m2")
            nc.vector.tensor_scalar(out=m2[:], in0=z2[:], scalar1=0.0,
                                    scalar2=None, op0=ALU.not_equal)
            if debug:
                nc.sync.dma_start(out=probe("z2", [64, 64])[:], in_=z2[:])

            # ---- Hopfield #1 -> D*m2 (factor 2 folded into w2b)
            q_ps = _hopfield(nc, sb, ps, z2[:], KT_bf[:], KV, ident, "1")
            qm = sb.tile([64, 64], F32, tag="qm")
            nc.vector.tensor_tensor(out=qm[:], in0=q_ps[:], in1=m2[:], op=ALU.mult)
            Dm2 = sb.tile([64, 64], F32, tag="Dm2")
            nc.vector.tensor_tensor(out=Dm2[:], in0=z2[:], in1=qm[:],
                                    op=ALU.subtract)

            # ---- backward: g1m = (w2b^T @ Dm2) * M1W, per chunk
            g1m = sb.tile([128, 4, 64], F32, tag="g1m")
            for t in range(4):
                g1_ps = ps.tile([128, 64], F32, tag="g128", bufs=4,
                                name=f"g1{t}")
                nc.tensor.matmul(g1_ps[:], w2b[:, t * 128:(t + 1) * 128],
                                 Dm2[:], start=True, stop=True)
                nc.vector.tensor_tensor(out=g1m[:, t, :], in0=g1_ps[:],
                                        in1=M1W[:, t, :], op=ALU.mult)

            # ---- C [100, 64] = sum_t Scomb_t^T @ g1m_t
            C_ps = ps.tile([100, 64], F32, tag="a1", bufs=1)
            for t in range(4):
                nc.tensor.matmul(C_ps[:], Scomb[:, t, :], g1m[:, t, :],
                                 start=(t == 0), stop=(t == 3))
            nc.scalar.copy(out=C_ext[0:100, :], in_=C_ps[:])
            C_sb = C_ext[0:100, :]
            if debug:
                nc.sync.dma_start(out=probe("C", [100, 64])[:], in_=C_sb)

            # ---- e_min dance
            cand = [None] * 3
            for cc in range(3):
                cand[cc] = ps.tile([128, 8, 8], F32, tag="g128", bufs=4,
                                   name=f"cand{cc}")
                nc.tensor.matmul(
                    cand[cc][:].rearrange("k p q -> k (p q)"),
                    CandM[:, cc, :], C_ext[:], start=True, stop=True)
            # shift-aligned candidate stack (zero prefill = min-with-0
            # candidate); one innermost-axis min-reduce collapses 8 classes.
            eB = sb.tile([16, 12, 8], F32, tag="eB")
            nc.vector.memset(eB[:], 0.0)
            cstk = sb.tile([16, 8, 8, 8], F32, tag="cstk")
            nc.gpsimd.memset(cstk[:], 0.0)
            for j, k in enumerate([0, 1, 2, 3, 5, 6, 7, 8]):
                cc, kk = divmod(k, 4)
                dp, dq = k // 3 - 1, k % 3 - 1
                i4lo, i4hi = max(0, dp), min(8, 8 + dp)
                j4lo, j4hi = max(0, dq), min(8, 8 + dq)
                srcap = cand[cc][kk * 32:kk * 32 + 16,
                                 i4lo - dp:i4hi - dp,
                                 j4lo - dq:j4hi - dq, None]
                dstap = cstk[:, i4lo:i4hi, j4lo:j4hi, j:j + 1]
                if k % 2 == 0:
                    nc.scalar.copy(out=dstap, in_=srcap)
                else:
                    nc.vector.tensor_copy(out=dstap, in_=srcap)
            nc.vector.tensor_reduce(out=eB[:, 2:10, :], in_=cstk[:],
                                    axis=mybir.AxisListType.X, op=ALU.min)
            # min with 0 (uncovered windows) and the full-range center class
            nc.vector.tensor_scalar_min(eB[:, 2:10, :], eB[:, 2:10, :], 0.0)
            nc.vector.tensor_tensor(out=eB[:, 2:10, :], in0=eB[:, 2:10, :],
                                    in1=cand[1][0:16, :, :], op=ALU.min)
            if debug:
                nc.sync.dma_start(out=probe("eB", [16, 96])[:],
                                  in_=eB[:].rearrange("a b c -> a (b c)"))
            eBf = eB[:].rearrange("a b c -> a (b c)")
            eW_ps = ps.tile([100, 64], F32, tag="S", bufs=1)
            for k in range(9):
                dp, dq = k // 3 - 1, k % 3 - 1
                off = 16 + 8 * dp + dq
                nc.tensor.matmul(eW_ps[:], PermB[:, k * 100:(k + 1) * 100],
                                 eBf[:, off:off + 64],
                                 start=(k == 0), stop=(k == 8))
            eW_sb = sb.tile([100, 64], F32, tag="eW_sb")
            nc.gpsimd.tensor_copy(out=eW_sb[:], in_=eW_ps[:]) if False else None
            maskw = sb.tile([100, 64], F32, tag="maskw")
            nc.vector.tensor_tensor(out=maskw[:], in0=C_sb, in1=eW_ps[:],
                                    op=ALU.is_le)
            if debug:
                nc.vector.tensor_copy(out=eW_sb[:], in_=eW_ps[:])
                nc.sync.dma_start(out=probe("eW", [100, 64])[:], in_=eW_sb[:])
            if debug:
                nc.sync.dma_start(out=probe("maskw", [100, 64])[:], in_=maskw[:])

            # ---- masked forward: Xm = X * maskw (broadcast over h)
            Xm = sb.tile([100, 3, 64], F32, tag="Xm")
            mask_b = bass.AP(tensor=maskw[:].tensor, offset=maskw[:].offset,
                             ap=[[64, 100], [0, 3], [1, 64]])
            nc.vector.tensor_tensor(out=Xm[:], in0=X, in1=mask_b, op=ALU.mult)
            u1m = sb.tile([128, 4, 64], F32, tag="u1m")
            for t in range(4):
                u1_ps = ps.tile([128, 64], F32, tag="g128", bufs=4,
                                name=f"u1{t}")
                for h in range(3):
                    nc.tensor.matmul(u1_ps[:], W1big[:, h, t, :], Xm[:, h, :],
                                     start=(h == 0), stop=(h == 2))
                nc.vector.tensor_tensor(out=u1m[:, t, :], in0=u1_ps[:],
                                        in1=M1W[:, t, :], op=ALU.mult)
            zm_ps = ps.tile([64, 64], F32, tag="q64", bufs=2)
            for t in range(4):
                nc.tensor.matmul(zm_ps[:], w2fT[:, t, :], u1m[:, t, :],
                                 start=(t == 0), stop=(t == 3))
            z2m = sb.tile([64, 64], F32, tag="z2m")
            nc.vector.tensor_tensor(out=z2m[:], in0=zm_ps[:], in1=m2[:],
                                    op=ALU.mult)
            if debug:
                nc.sync.dma_start(out=probe("z2m", [64, 64])[:], in_=z2m[:])

            # ---- Hopfield #2 -> output
            q2_ps = _hopfield(nc, sb, ps, z2m[:], KT_bf[:], KV, ident, "2")
            out_sb = sb.tile([64, 64], F32, tag="out_sb")
            nc.vector.tensor_copy(out=out_sb[:], in_=q2_ps[:])
            nc.sync.dma_start(out=out_t[:], in_=out_sb[:])
    nc.compile()
    return nc


def _get_nc(debug=False):
    key = ("nc", debug)
    if key not in _CACHE:
        _CACHE[key] = _build_nc(debug)
    return _CACHE[key]


# ---------------------------------------------------------------- entry point
def kernel(x, w1, b1, w2, b2, K, Vw, _debug=False):
    x = np.asarray(x, np.float32)
    shared = _host_prep(np.asarray(w1, np.float32), np.asarray(b1, np.float32),
                        np.asarray(w2, np.float32), np.asarray(b2, np.float32),
                        np.asarray(K, np.float32), np.asarray(Vw, np.float32))
    w1f, b1c = shared.pop("_w1f"), shared.pop("_b1")
    bsz = x.shape[0]
    nc = _get_nc(_debug)
    smpls = [_sample_prep(x[b], w1f, b1c) for b in range(bsz)]
    in_maps = []
    for core in range(N_CORES):
        m = dict(shared)
        m["smpl"] = smpls[core] if core < bsz else smpls[0]
        in_maps.append(m)
    res = run_bass_kernel_spmd(nc, in_maps, core_ids=list(range(N_CORES)))
    out = np.stack([res.results[b]["out"].reshape(64, 8, 8)
                    for b in range(bsz)]).astype(np.float32)
    if _debug:
        return out, res
    return out


# revision 17
# speedup vs baseline: 1.0086x; 1.0086x over previous
"""TRN2 Bass kernel for nn_Block1_43542378447225.

Pipeline (per sample, one NeuronCore; batch=2 -> cores 0/1 do real work):
  conv1 -> relu -> conv2 -> relu -> Hopfield(z2) -> D
  backward (w2b matmul, mask, Scomb matmul) -> C  (e_sum in composite-window form)
  blocked e_min via permutation matmuls -> mask -> masked patch forward -> z2_masked
  Hopfield(z2_masked) -> output

Host precomputes im2col patches (P1 [48,256], X [100,192]) and weight layouts,
packed into few DMA-able blocks; the device does all matmuls/softmax/masking.

Layout conventions:
  pq = p*8+q (64 output positions), uv = u*10+v (100 composite-window offsets)
  chunk t = conv2 kernel row kr, a = conv2 kernel col ks
  kc = t*128 + a*32 + c1 (hidden index, 4 chunks of 128 partitions)
"""
import numpy as np

import concourse.bass as bass
import concourse.bacc as bacc
import concourse.mybir as mybir
import concourse.tile as tile
from concourse.bass_utils import run_bass_kernel_spmd

F32 = mybir.dt.float32
BF16 = mybir.dt.bfloat16
AF = mybir.ActivationFunctionType
ALU = mybir.AluOpType

N_CORES = 8
BETA = 0.125  # 1/sqrt(64)
BIG = 1.0e30

_CACHE = {}


# ---------------------------------------------------------------- host prep
def _build_scomb_w1big(w1):
    w1s = w1.sum(axis=1)
    Scomb = np.zeros((4, 32, 4, 100), np.float32)  # [a, c1, t, uv]
    W1big = np.zeros((100, 3, 4, 4, 32), np.float32)  # [uv, h, t, a, c1]
    for t in range(4):
        for a in range(4):
            for u in range(10):
                ki = u - 2 * t
                if not (0 <= ki < 4):
                    continue
                for v in range(10):
                    kj = v - 2 * a
                    if not (0 <= kj < 4):
                        continue
                    Scomb[a, :, t, u * 10 + v] = w1s[:, ki, kj]
                    W1big[u * 10 + v, :, t, a, :] = w1[:, :, ki, kj].T
    # partition index = a*32+c1 -> merge (a, c1); free = t*100+uv
    Scomb = Scomb.reshape(128, 400)
    W1big = W1big.reshape(100, 1536)
    return Scomb, W1big


def _host_prep(w1, b1, w2, b2, K, Vw):
    # wA [128, 897]: w2fT | b2 | KT | Vw | ident
    wA = np.zeros((128, 897), np.float32)
    wA[:, 0:256] = np.transpose(w2, (3, 1, 2, 0)).reshape(128, 256)  # w2fT
    wA[0:64, 256:257] = b2[:, None]
    wA[0:64, 257:769] = K.T
    wA[0:64, 769:833] = Vw
    wA[0:64, 833:897] = np.eye(64, dtype=np.float32)

    Scomb, W1big = _build_scomb_w1big(w1)
    PermF = np.zeros((100, 9, 16), np.float32)
    for k in range(9):
        dp, dq = k // 3 - 1, k % 3 - 1
        for im in range(4):
            u = 4 * dp + im + 3
            if not (0 <= u < 10):
                continue
            for jm in range(4):
                v = 4 * dq + jm + 3
                if not (0 <= v < 10):
                    continue
                PermF[u * 10 + v, k, im * 4 + jm] = 1.0
    CandM = np.zeros((100, 3, 128), np.float32)
    for k in range(9):
        cc, kk = divmod(k, 4)
        CandM[:, cc, kk * 32:kk * 32 + 16] = PermF[:, k, :]
    PermB = np.transpose(PermF, (2, 1, 0)).reshape(16, 900)

    # wB [128, 2196]: w2b | Scomb | CandM_ext | PermB
    wB = np.zeros((128, 2196), np.float32)
    wB[0:64, 0:512] = 2.0 * np.transpose(w2, (0, 2, 3, 1)).reshape(64, 512)
    wB[:, 512:912] = Scomb
    wB[0:100, 912:1296] = CandM.reshape(100, 384)
    wB[0:16, 1296:2196] = PermB

    # wC [128, 1536]: W1big rows 0:100
    wC = np.zeros((128, 1536), np.float32)
    wC[0:100, :] = W1big

    return {"wA": wA, "wB": wB, "wC": wC,
            "_w1f": np.ascontiguousarray(np.transpose(w1, (2, 3, 1, 0)).reshape(48, 32)),
            "_b1": np.ascontiguousarray(b1[:, None])}


def _sample_prep(x_s, w1f, b1c):
    xp1 = np.pad(x_s, ((0, 0), (1, 1), (1, 1)))
    xp3 = np.pad(x_s, ((0, 0), (3, 3), (3, 3)))
    P1 = np.zeros((4, 4, 3, 16, 16), np.float32)
    for kr in range(4):
        for ks in range(4):
            P1[kr, ks] = xp1[:, kr:kr + 32:2, ks:ks + 32:2][:, :16, :16]
    X = np.zeros((10, 10, 3, 8, 8), np.float32)
    for u in range(10):
        for v in range(10):
            X[u, v] = xp3[:, u:u + 32:4, v:v + 32:4][:, :8, :8]
    cv = np.zeros((48, 289), np.float32)
    cv[:, 0:256] = P1.reshape(48, 256)
    cv[:, 256:288] = w1f
    cv[0:32, 288:289] = b1c
    return cv, X.reshape(100, 192).copy()


# ---------------------------------------------------------------- device build
def _hopfield(nc, sb, ps, z_sb, KT_bf, KV_sb, ident_sb, tag):
    """z_sb [64(c), 64(pq)] -> returns q_ps [64(c), 64(pq)] PSUM tile.
    Scores are bounded (|beta*S| small), so softmax skips max subtraction."""
    z_bf = sb.tile([64, 64], BF16, tag=f"zbf{tag}", name=f"zbf{tag}")
    nc.vector.tensor_copy(out=z_bf[:], in_=z_sb)
    S_ps = ps.tile([64, 512], F32, tag="S", bufs=1, name=f"S{tag}")
    nc.tensor.matmul(S_ps[:], z_bf[:], KT_bf, start=True, stop=True)
    att = sb.tile([64, 512], F32, tag=f"att{tag}", name=f"att{tag}")
    ssum = sb.tile([64, 1], F32, tag=f"ssum{tag}", name=f"ssum{tag}")
    nc.scalar.activation(out=att[:], in_=S_ps[:], func=AF.Exp,
                         bias=0.0, scale=BETA, accum_out=ssum[:])
    rec = sb.tile([64, 1], F32, tag=f"rec{tag}", name=f"rec{tag}")
    nc.vector.reciprocal(rec[:], ssum[:])
    nc.vector.tensor_scalar_mul(att[:], att[:], rec[:])
    attT = sb.tile([128, 4, 64], F32, tag=f"attT{tag}", name=f"attT{tag}")
    for t in range(4):
        tr_ps = ps.tile([128, 64], F32, tag="g128", bufs=4, name=f"tr{tag}{t}")
        nc.tensor.transpose(tr_ps[:], att[:, t * 128:(t + 1) * 128], ident_sb)
        if t % 2 == 0:
            nc.vector.tensor_copy(out=attT[:, t, :], in_=tr_ps[:])
        else:
            nc.scalar.copy(out=attT[:, t, :], in_=tr_ps[:])
    q_ps = ps.tile([64, 64], F32, tag="q64", bufs=2, name=f"q{tag}")
    for t in range(4):
        nc.tensor.matmul(q_ps[:], KV_sb[:, t, :], attT[:, t, :],
                         start=(t == 0), stop=(t == 3))
    return q_ps


def _build_nc(debug=False):
    nc = bacc.Bacc("TRN2", target_bir_lowering=False, debug=False,
                   num_devices=N_CORES)
    d_cv = nc.dram_tensor("cv", [48, 289], F32, kind="ExternalInput")
    d_smpl = nc.dram_tensor("smpl", [100, 192], F32, kind="ExternalInput")
    d_wA = nc.dram_tensor("wA", [128, 897], F32, kind="ExternalInput")
    d_wB = nc.dram_tensor("wB", [128, 2196], F32, kind="ExternalInput")
    d_wC = nc.dram_tensor("wC", [128, 1536], F32, kind="ExternalInput")
    out_t = nc.dram_tensor("out", [64, 64], F32, kind="ExternalOutput")
    probes = {}

    def probe(name, shape):
        if debug:
            probes[name] = nc.dram_tensor("probe_" + name, shape, F32,
                                          kind="ExternalOutput")
        return probes.get(name)

    with tile.TileContext(nc) as tc:
        with tc.tile_pool(name="sb", bufs=1) as sb, \
             tc.tile_pool(name="ps", bufs=1, space="PSUM") as ps:
            # ---- loads: 2 HWDGE queues (SP: cv+smpl+wB, ACT: wA+wC)
            cv = sb.tile([48, 289], F32, tag="cv")
            nc.sync.dma_start(out=cv[:], in_=d_cv[:])
            wA = sb.tile([128, 897], F32, tag="wA")
            nc.scalar.dma_start(out=wA[:], in_=d_wA[:])
            smpl = sb.tile([100, 192], F32, tag="smpl")
            nc.sync.dma_start(out=smpl[:], in_=d_smpl[:])
            wB = sb.tile([128, 2196], F32, tag="wB")
            nc.sync.dma_start(out=wB[:], in_=d_wB[:])
            wC = sb.tile([128, 1536], F32, tag="wC")
            nc.scalar.dma_start(out=wC[:], in_=d_wC[:])
            C_ext = sb.tile([100, 64], F32, tag="C_ext")

            P1 = cv[:, 0:256]
            X = smpl[:].rearrange("u (h q) -> u h q", h=3)
            w1f = cv[:, 256:288]
            b1 = cv[0:32, 288:289]
            w2fT = wA[:, 0:256].rearrange("k (t c) -> k t c", t=4)
            b2 = wA[0:64, 256:257]
            KT = wA[0:64, 257:769]
            Vw = wA[0:64, 769:833]
            ident = wA[0:64, 833:897]
            w2b = wB[0:64, 0:512]
            Scomb = wB[:, 512:912].rearrange("k (t u) -> k t u", t=4)
            CandM = wB[0:100, 912:1296].rearrange("u (c k) -> u c k", c=3)
            PermB = wB[0:16, 1296:2196]
            W1big = wC[0:100, :].rearrange("u (h t k) -> u h t k", h=3, t=4)

            # ---- conv1 + relu into padded a1p [32, 18, 18]
            a1_ps = ps.tile([32, 256], F32, tag="a1", bufs=1)
            nc.tensor.matmul(a1_ps[:], w1f, P1, start=True, stop=True)
            a1p = sb.tile([32, 18, 18], F32, tag="a1p")
            nc.vector.memset(a1p[:], 0.0)
            nc.scalar.activation(
                out=a1p[:, 1:17, 1:17],
                in_=a1_ps[:].rearrange("c (p q) -> c p q", p=16),
                func=AF.Relu, bias=b1, scale=1.0)

            # ---- KV = K @ Vw chunked [128, 4, 64]; KT in bf16 for scores
            KT_bf = sb.tile([64, 512], BF16, tag="KT_bf")
            nc.vector.tensor_copy(out=KT_bf[:], in_=KT)
            KV = sb.tile([128, 4, 64], F32, tag="KV")
            for t in range(4):
                kv_ps = ps.tile([128, 64], F32, tag="g128", bufs=4,
                                name=f"kv{t}")
                nc.tensor.matmul(kv_ps[:], KT[:, t * 128:(t + 1) * 128],
                                 Vw, start=True, stop=True)
                nc.scalar.copy(out=KV[:, t, :], in_=kv_ps[:])

            # ---- P2 im2col: P2[a*32+c1, t, p, q] = a1p[c1, 2p+t, 2q+a]
            P2 = sb.tile([128, 4, 8, 8], F32, tag="P2")
            a1p_ap = a1p[:]
            for a in range(4):
                src = bass.AP(
                    tensor=a1p_ap.tensor,
                    offset=a1p_ap.offset + a,
                    ap=[[324, 32], [18, 4], [36, 8], [2, 8]])
                eng = nc.vector if a < 3 else nc.gpsimd
                eng.tensor_copy(out=P2[a * 32:(a + 1) * 32, :, :, :], in_=src)
            P2f = P2[:].rearrange("k t p q -> k t (p q)")
            M1W = sb.tile([128, 4, 64], F32, tag="M1W")
            nc.vector.tensor_scalar(out=M1W[:], in0=P2f, scalar1=0.0,
                                    scalar2=None, op0=ALU.not_equal)

            # ---- conv2 + relu -> z2 [64, 64], m2
            z2_ps = ps.tile([64, 64], F32, tag="q64", bufs=2)
            for t in range(4):
                nc.tensor.matmul(z2_ps[:], w2fT[:, t, :], P2f[:, t, :],
                                 start=(t == 0), stop=(t == 3))
            z2 = sb.tile([64, 64], F32, tag="z2")
            nc.scalar.activation(out=z2[:], in_=z2_ps[:], func=AF.Relu,
                                 bias=b2, scale=1.0)
            m2 = sb.tile([64, 64], F32, tag="m2")
            nc.vector.tensor_scalar(out=m2[:], in0=z2[:], scalar1=0.0,
                                    scalar2=None, op0=ALU.not_equal)
            if debug:
                nc.sync.dma_start(out=probe("z2", [64, 64])[:], in_=z2[:])

            # ---- Hopfield #1 -> D*m2 (factor 2 folded into w2b)
            q_ps = _hopfield(nc, sb, ps, z2[:], KT_bf[:], KV, ident, "1")
            qm = sb.tile([64, 64], F32, tag="qm")
            nc.vector.tensor_tensor(out=qm[:], in0=q_ps[:], in1=m2[:], op=ALU.mult)
            Dm2 = sb.tile([64, 64], F32, tag="Dm2")
            nc.vector.tensor_tensor(out=Dm2[:], in0=z2[:], in1=qm[:],
                                    op=ALU.subtract)

            # ---- backward: g1m = (w2b^T @ Dm2) * M1W, per chunk
            g1m = sb.tile([128, 4, 64], F32, tag="g1m")
            for t in range(4):
                g1_ps = ps.tile([128, 64], F32, tag="g128", bufs=4,
                                name=f"g1{t}")
                nc.tensor.matmul(g1_ps[:], w2b[:, t * 128:(t + 1) * 128],
                                 Dm2[:], start=True, stop=True)
                nc.vector.tensor_tensor(out=g1m[:, t, :], in0=g1_ps[:],
                                        in1=M1W[:, t, :], op=ALU.mult)

            # ---- C [100, 64] = sum_t Scomb_t^T @ g1m_t
            C_ps = ps.tile([100, 64], F32, tag="a1", bufs=1)
            for t in range(4):
                nc.tensor.matmul(C_ps[:], Scomb[:, t, :], g1m[:, t, :],
                                 start=(t == 0), stop=(t == 3))
            nc.scalar.copy(out=C_ext[0:100, :], in_=C_ps[:])
            C_sb = C_ext[0:100, :]
            if debug:
                nc.sync.dma_start(out=probe("C", [100, 64])[:], in_=C_sb)

            # ---- e_min dance
            cand = [None] * 3
            for cc in range(3):
                cand[cc] = ps.tile([128, 8, 8], F32, tag="g128", bufs=4,
                                   name=f"cand{cc}")
                nc.tensor.matmul(
                    cand[cc][:].rearrange("k p q -> k (p q)"),
                    CandM[:, cc, :], C_ext[:], start=True, stop=True)
            # shift-aligned candidate stack (zero prefill = min-with-0
            # candidate); one innermost-axis min-reduce collapses 8 classes.
            eB = sb.tile([16, 12, 8], F32, tag="eB")
            nc.vector.memset(eB[:], 0.0)
            cstk = sb.tile([16, 8, 8, 10], F32, tag="cstk")
            nc.gpsimd.memset(cstk[:], 0.0)
            for j, k in enumerate([0, 1, 2, 3, 5, 6, 7, 8]):
                cc, kk = divmod(k, 4)
                dp, dq = k // 3 - 1, k % 3 - 1
                i4lo, i4hi = max(0, dp), min(8, 8 + dp)
                j4lo, j4hi = max(0, dq), min(8, 8 + dq)
                srcap = cand[cc][kk * 32:kk * 32 + 16,
                                 i4lo - dp:i4hi - dp,
                                 j4lo - dq:j4hi - dq, None]
                dstap = cstk[:, i4lo:i4hi, j4lo:j4hi, j:j + 1]
                if k % 2 == 0:
                    nc.scalar.copy(out=dstap, in_=srcap)
                else:
                    nc.vector.tensor_copy(out=dstap, in_=srcap)
            nc.scalar.copy(out=cstk[:, :, :, 8:9],
                           in_=cand[1][0:16, :, :, None])
            nc.vector.tensor_reduce(out=eB[:, 2:10, :], in_=cstk[:],
                                    axis=mybir.AxisListType.X, op=ALU.min)
            if debug:
                nc.sync.dma_start(out=probe("eB", [16, 96])[:],
                                  in_=eB[:].rearrange("a b c -> a (b c)"))
            eBf = eB[:].rearrange("a b c -> a (b c)")
            eW_ps = ps.tile([100, 64], F32, tag="S", bufs=1)
            for k in range(9):
                dp, dq = k // 3 - 1, k % 3 - 1
                off = 16 + 8 * dp + dq
                nc.tensor.matmul(eW_ps[:], PermB[:, k * 100:(k + 1) * 100],
                                 eBf[:, off:off + 64],
                                 start=(k == 0), stop=(k == 8))
            eW_sb = sb.tile([100, 64], F32, tag="eW_sb")
            nc.gpsimd.tensor_copy(out=eW_sb[:], in_=eW_ps[:]) if False else None
            maskw = sb.tile([100, 64], F32, tag="maskw")
            nc.vector.tensor_tensor(out=maskw[:], in0=C_sb, in1=eW_ps[:],
                                    op=ALU.is_le)
            if debug:
                nc.vector.tensor_copy(out=eW_sb[:], in_=eW_ps[:])
                nc.sync.dma_start(out=probe("eW", [100, 64])[:], in_=eW_sb[:])
            if debug:
                nc.sync.dma_start(out=probe("maskw", [100, 64])[:], in_=maskw[:])

            # ---- masked forward: Xm = X * maskw (broadcast over h)
            Xm = sb.tile([100, 3, 64], F32, tag="Xm")
            mask_b = bass.AP(tensor=maskw[:].tensor, offset=maskw[:].offset,
                             ap=[[64, 100], [0, 3], [1, 64]])
            nc.vector.tensor_tensor(out=Xm[:], in0=X, in1=mask_b, op=ALU.mult)
            u1m = sb.tile([128, 4, 64], F32, tag="u1m")
            for t in range(4):
                u1_ps = ps.tile([128, 64], F32, tag="g128", bufs=4,
                                name=f"u1{t}")
                for h in range(3):
                    nc.tensor.matmul(u1_ps[:], W1big[:, h, t, :], Xm[:, h, :],
                                     start=(h == 0), stop=(h == 2))
                nc.vector.tensor_tensor(out=u1m[:, t, :], in0=u1_ps[:],
                                        in1=M1W[:, t, :], op=ALU.mult)
            zm_ps = ps.tile([64, 64], F32, tag="q64", bufs=2)
            for t in range(4):
                nc.tensor.matmul(zm_ps[:], w2fT[:, t, :], u1m[:, t, :],
                                 start=(t == 0), stop=(t == 3))
            z2m = sb.tile([64, 64], F32, tag="z2m")
            nc.vector.tensor_tensor(out=z2m[:], in0=zm_ps[:], in1=m2[:],
                                    op=ALU.mult)
            if debug:
                nc.sync.dma_start(out=probe("z2m", [64, 64])[:], in_=z2m[:])

            # ---- Hopfield #2 -> output
            q2_ps = _hopfield(nc, sb, ps, z2m[:], KT_bf[:], KV, ident, "2")
            out_sb = sb.tile([64, 64], F32, tag="out_sb")
            nc.vector.tensor_copy(out=out_sb[:], in_=q2_ps[:])
            nc.sync.dma_start(out=out_t[:], in_=out_sb[:])
    nc.compile()
    return nc


def _get_nc(debug=False):
    key = ("nc", debug)
    if key not in _CACHE:
        _CACHE[key] = _build_nc(debug)
    return _CACHE[key]


# ---------------------------------------------------------------- entry point
def kernel(x, w1, b1, w2, b2, K, Vw, _debug=False):
    x = np.asarray(x, np.float32)
    shared = _host_prep(np.asarray(w1, np.float32), np.asarray(b1, np.float32),
                        np.asarray(w2, np.float32), np.asarray(b2, np.float32),
                        np.asarray(K, np.float32), np.asarray(Vw, np.float32))
    w1f, b1c = shared.pop("_w1f"), shared.pop("_b1")
    bsz = x.shape[0]
    nc = _get_nc(_debug)
    smpls = [_sample_prep(x[b], w1f, b1c) for b in range(bsz)]
    in_maps = []
    for core in range(N_CORES):
        cvb, xb = smpls[core] if core < bsz else smpls[0]
        m = dict(shared)
        m["cv"], m["smpl"] = cvb, xb
        in_maps.append(m)
    res = run_bass_kernel_spmd(nc, in_maps, core_ids=list(range(N_CORES)))
    out = np.stack([res.results[b]["out"].reshape(64, 8, 8)
                    for b in range(bsz)]).astype(np.float32)
    if _debug:
        return out, res
    return out
